# revision 1
# baseline (speedup 1.0000x reference)
"""LoRA QKV parallel linear with per-token slot routing, on 8 TRN2 NeuronCores.

Strategy: data-parallel over the token dim T (8192 -> 1024 tokens/core),
weights replicated. Per core, everything is one fused pass:

  y[t, o] = sum_k x[t,k] W[o,k]  +  sum_{l,r} mask[t,l] * h[t, g(o), l, r] * B_g[l, o, r]

where h = x @ A^T for all 8 slots (dense; 384 extra contraction cols is ~19%
more FLOPs than routing but fully regular), and the routing is applied as an
on-device one-hot mask (is_equal against a constant slot-id tile). The LoRA
scaling is folded into the B matrices host-side.

All matmul operands are pre-transposed host-side so the contraction dim K is
on SBUF partitions, and use float32r (full-rate fp32 via transpose mode).
"""

import numpy as np

import concourse.bass as bass
import concourse.bacc as bacc
import concourse.mybir as mybir
import concourse.tile as tile
from concourse.masks import make_identity

HIDDEN = 2048
Q_SIZE = 2048
KV_SIZE = 512
OUT = Q_SIZE + 2 * KV_SIZE  # 3072
MAX_LORAS = 8
RANK = 16
T = 8192
N_CORES = 8
T_CORE = T // N_CORES  # 1024

P = 128
NT = T_CORE // P          # 8 token tiles per core
KC = HIDDEN // P          # 16 k-chunks
OJ = OUT // 512           # 6 output chunks of 512
GR = MAX_LORAS * RANK     # 128 = all slots*ranks for one target group
F32 = mybir.dt.float32
F32R = mybir.dt.float32r

_NC_CACHE = {}


def build_nc(mm_dtype=F32R):
    """Build the SPMD Bass program (same program on every core)."""
    nc = bacc.Bacc("TRN2", target_bir_lowering=False, debug=False, num_devices=N_CORES)

    xT = nc.dram_tensor("xT", [HIDDEN, T_CORE], F32R, kind="ExternalInput").ap()
    wT = nc.dram_tensor("wT", [HIDDEN, OUT], F32R, kind="ExternalInput").ap()
    aT = nc.dram_tensor("aT", [HIDDEN, 3 * GR], F32R, kind="ExternalInput").ap()
    bq = nc.dram_tensor("bq", [GR, Q_SIZE], F32R, kind="ExternalInput").ap()
    bk = nc.dram_tensor("bk", [GR, KV_SIZE], F32R, kind="ExternalInput").ap()
    bv = nc.dram_tensor("bv", [GR, KV_SIZE], F32R, kind="ExternalInput").ap()
    slotf = nc.dram_tensor("slotf", [T_CORE, 1], F32, kind="ExternalInput").ap()
    y = nc.dram_tensor("y", [T_CORE, OUT], F32, kind="ExternalOutput").ap()

    with tile.TileContext(nc) as tc:
        with (
            tc.tile_pool(name="const", bufs=1) as const_pool,
            tc.tile_pool(name="xsb", bufs=1) as xpool,
            tc.tile_pool(name="asb", bufs=1) as apool,
            tc.tile_pool(name="bsb", bufs=1) as bpool,
            tc.tile_pool(name="hT", bufs=1) as htpool,
            tc.tile_pool(name="m", bufs=2) as mpool,
            tc.tile_pool(name="hm", bufs=2) as hmpool,
            tc.tile_pool(name="w", bufs=3) as wpool,
            tc.tile_pool(name="o", bufs=4) as opool,
            tc.tile_pool(name="hps", bufs=2, space="PSUM") as hpsum,
            tc.tile_pool(name="tps", bufs=2, space="PSUM") as tpsum,
            tc.tile_pool(name="ops", bufs=4, space="PSUM") as opsum,
        ):
            # --- constants ---
            ident = const_pool.tile([P, P], F32)
            make_identity(nc, ident)
            c128 = const_pool.tile([P, P], F32)  # c128[t, l*16+r] = l
            for l in range(MAX_LORAS):
                nc.vector.memset(c128[:, l * RANK:(l + 1) * RANK], float(l))
            slot_sb = const_pool.tile([P, NT], F32)  # col i = token tile i
            nc.sync.dma_start(
                slot_sb[:], slotf.rearrange("(n p) o -> p (n o)", p=P))

            # --- resident inputs ---
            xsb = xpool.tile([P, KC * T_CORE], F32R)  # free idx = k*T_CORE + t
            for k in range(KC):
                nc.sync.dma_start(
                    xsb[:, k * T_CORE:(k + 1) * T_CORE],
                    xT[k * P:(k + 1) * P, :])
            asb = apool.tile([P, KC * 3 * GR], F32R)  # free idx = k*384 + (g*128+l*16+r)
            for k in range(KC):
                nc.sync.dma_start(
                    asb[:, k * 3 * GR:(k + 1) * 3 * GR],
                    aT[k * P:(k + 1) * P, :])
            bqsb = bpool.tile([P, Q_SIZE], F32R)
            bksb = bpool.tile([P, KV_SIZE], F32R)
            bvsb = bpool.tile([P, KV_SIZE], F32R)
            nc.sync.dma_start(bqsb[:], bq[:, :])
            nc.sync.dma_start(bksb[:], bk[:, :])
            nc.sync.dma_start(bvsb[:], bv[:, :])

            # --- phase 1: h = x@A^T per token tile, mask, transpose ---
            # hT_sb[:, i*384 + g*128 : ...] = (mask * h_g)^T for token tile i
            hT_sb = htpool.tile([P, NT * 3 * GR], F32R)
            for i in range(NT):
                hps = hpsum.tile([P, 3 * GR], F32)
                for k in range(KC):
                    nc.tensor.matmul(
                        hps[:],
                        lhsT=xsb[:, k * T_CORE + i * P: k * T_CORE + (i + 1) * P],
                        rhs=asb[:, k * 3 * GR:(k + 1) * 3 * GR],
                        start=(k == 0), stop=(k == KC - 1))
                m128 = mpool.tile([P, P], F32)  # one-hot routing mask
                nc.vector.tensor_scalar(
                    m128[:], c128[:], slot_sb[:, i:i + 1], None,
                    op0=mybir.AluOpType.is_equal)
                hm = hmpool.tile([P, 3 * GR], F32)
                for g in range(3):
                    nc.vector.tensor_tensor(
                        hm[:, g * GR:(g + 1) * GR],
                        hps[:, g * GR:(g + 1) * GR], m128[:],
                        op=mybir.AluOpType.mult)
                for g in range(3):
                    tp = tpsum.tile([P, P], F32)
                    nc.tensor.transpose(tp[:], hm[:, g * GR:(g + 1) * GR], ident[:])
                    nc.vector.tensor_copy(
                        hT_sb[:, (i * 3 + g) * GR:(i * 3 + g + 1) * GR], tp[:])

            # --- phase 2: y = x@W^T + hm@B, streamed over 512-wide o-chunks ---
            for j in range(OJ):
                if j < Q_SIZE // 512:
                    g, bsl = 0, bqsb[:, j * 512:(j + 1) * 512]
                elif j == Q_SIZE // 512:
                    g, bsl = 1, bksb[:]
                else:
                    g, bsl = 2, bvsb[:]
                wh = []
                for half in range(2):
                    wsb = wpool.tile([P, 8 * 512], F32R, tag="w")
                    for kk in range(8):
                        k = half * 8 + kk
                        nc.sync.dma_start(
                            wsb[:, kk * 512:(kk + 1) * 512],
                            wT[k * P:(k + 1) * P, j * 512:(j + 1) * 512])
                    wh.append(wsb)
                for i in range(NT):
                    ops = opsum.tile([P, 512], F32)
                    for k in range(KC):
                        nc.tensor.matmul(
                            ops[:],
                            lhsT=xsb[:, k * T_CORE + i * P: k * T_CORE + (i + 1) * P],
                            rhs=wh[k // 8][:, (k % 8) * 512:(k % 8 + 1) * 512],
                            start=(k == 0), stop=False)
                    nc.tensor.matmul(
                        ops[:],
                        lhsT=hT_sb[:, (i * 3 + g) * GR:(i * 3 + g + 1) * GR],
                        rhs=bsl,
                        start=False, stop=True)
                    osb = opool.tile([P, 512], F32)
                    nc.scalar.copy(osb[:], ops[:])
                    nc.sync.dma_start(
                        y[i * P:(i + 1) * P, j * 512:(j + 1) * 512], osb[:])
    nc.compile()
    return nc


def prep_in_maps(x, weight, lora_A, lora_B_q, lora_B_k, lora_B_v,
                 lora_scaling, token_to_slot):
    x = np.asarray(x, dtype=np.float32)
    weight = np.asarray(weight, dtype=np.float32)
    lora_A = np.asarray(lora_A, dtype=np.float32)
    lora_B_q = np.asarray(lora_B_q, dtype=np.float32)
    lora_B_k = np.asarray(lora_B_k, dtype=np.float32)
    lora_B_v = np.asarray(lora_B_v, dtype=np.float32)
    lora_scaling = np.asarray(lora_scaling, dtype=np.float32)
    slot = np.asarray(token_to_slot)

    xT = np.ascontiguousarray(x.T)                      # (2048, 8192)
    wT = np.ascontiguousarray(weight.T)                 # (2048, 3072)
    # aT col = g*128 + l*16 + r
    aT = np.ascontiguousarray(
        lora_A.transpose(1, 0, 2, 3).reshape(3 * GR, HIDDEN).T)
    # b row = l*16 + r, with scaling folded in
    bq = np.ascontiguousarray(
        (lora_scaling[:, None, None] * lora_B_q).transpose(0, 2, 1).reshape(GR, Q_SIZE))
    bk = np.ascontiguousarray(
        (lora_scaling[:, None, None] * lora_B_k).transpose(0, 2, 1).reshape(GR, KV_SIZE))
    bv = np.ascontiguousarray(
        (lora_scaling[:, None, None] * lora_B_v).transpose(0, 2, 1).reshape(GR, KV_SIZE))
    slotf = slot.astype(np.float32).reshape(T, 1)

    in_maps = []
    for c in range(N_CORES):
        in_maps.append({
            "xT": np.ascontiguousarray(xT[:, c * T_CORE:(c + 1) * T_CORE]),
            "wT": wT,
            "aT": aT,
            "bq": bq,
            "bk": bk,
            "bv": bv,
            "slotf": np.ascontiguousarray(slotf[c * T_CORE:(c + 1) * T_CORE]),
        })
    return in_maps


def kernel(**inputs):
    from concourse.bass_utils import run_bass_kernel_spmd
    if "nc" not in _NC_CACHE:
        _NC_CACHE["nc"] = build_nc()
    nc = _NC_CACHE["nc"]
    in_maps = prep_in_maps(**inputs)
    res = run_bass_kernel_spmd(nc, in_maps, core_ids=list(range(N_CORES)))
    return np.concatenate([r["y"] for r in res.results], axis=0)



# revision 4
# speedup vs baseline: 1368.9879x; 1368.9879x over previous
"""LoRA QKV parallel linear with per-token slot routing, on 8 TRN2 NeuronCores.

Strategy: data-parallel over the token dim T (8192 -> 1024 tokens/core),
weights replicated. Per core, everything is one fused pass:

  y[t, o] = sum_k x[t,k] W[o,k]  +  sum_{l,r} mask[t,l] * h[t, g(o), l, r] * B_g[l, o, r]

where h = x @ A^T for all 8 slots (dense; 384 extra contraction cols is ~19%
more FLOPs than routing but fully regular), and the routing is applied as an
on-device one-hot mask (is_equal against a constant slot-id tile). The LoRA
scaling is folded into the B matrices host-side.

All matmul operands are pre-transposed host-side so the contraction dim K is
on SBUF partitions, and use float32r (full-rate fp32 via transpose mode).
"""

import numpy as np

import concourse.bass as bass
import concourse.bacc as bacc
import concourse.mybir as mybir
import concourse.tile as tile
from concourse.masks import make_identity

HIDDEN = 2048
Q_SIZE = 2048
KV_SIZE = 512
OUT = Q_SIZE + 2 * KV_SIZE  # 3072
MAX_LORAS = 8
RANK = 16
T = 8192
N_CORES = 8
T_CORE = T // N_CORES  # 1024

P = 128
NT = T_CORE // P          # 8 token tiles per core
KC = HIDDEN // P          # 16 k-chunks
OJ = OUT // 512           # 6 output chunks of 512
GR = MAX_LORAS * RANK     # 128 = all slots*ranks for one target group
F32 = mybir.dt.float32
F32R = mybir.dt.float32r

_NC_CACHE = {}


def build_nc(mm_dtype=F32R, reps=1):
    """Build the SPMD Bass program (same program on every core)."""
    nc = bacc.Bacc("TRN2", target_bir_lowering=False, debug=False, num_devices=N_CORES)

    xT = nc.dram_tensor("xT", [HIDDEN, T_CORE], F32R, kind="ExternalInput").ap()
    wT = nc.dram_tensor("wT", [HIDDEN, OUT], F32R, kind="ExternalInput").ap()
    aT = nc.dram_tensor("aT", [HIDDEN, 3 * GR], F32R, kind="ExternalInput").ap()
    bq = nc.dram_tensor("bq", [GR, Q_SIZE], F32R, kind="ExternalInput").ap()
    bk = nc.dram_tensor("bk", [GR, KV_SIZE], F32R, kind="ExternalInput").ap()
    bv = nc.dram_tensor("bv", [GR, KV_SIZE], F32R, kind="ExternalInput").ap()
    slotf = nc.dram_tensor("slotf", [T_CORE, 1], F32, kind="ExternalInput").ap()
    if reps > 1:  # defeat NEFF cache collision between reps variants
        nc.dram_tensor(f"dummy_reps{reps}", [1, 1], F32, kind="ExternalInput")
    y = nc.dram_tensor("y", [T_CORE, OUT], F32, kind="ExternalOutput").ap()

    with tile.TileContext(nc) as tc:
      for _rep in range(reps):
        with (
            tc.tile_pool(name="const", bufs=1) as const_pool,
            tc.tile_pool(name="xsb", bufs=1) as xpool,
            tc.tile_pool(name="asb", bufs=1) as apool,
            tc.tile_pool(name="bsb", bufs=1) as bpool,
            tc.tile_pool(name="hT", bufs=1) as htpool,
            tc.tile_pool(name="m", bufs=2) as mpool,
            tc.tile_pool(name="hm", bufs=2) as hmpool,
            tc.tile_pool(name="w", bufs=3) as wpool,
            tc.tile_pool(name="o", bufs=4) as opool,
            tc.tile_pool(name="hps", bufs=2, space="PSUM") as hpsum,
            tc.tile_pool(name="tps", bufs=2, space="PSUM") as tpsum,
            tc.tile_pool(name="ops", bufs=4, space="PSUM") as opsum,
        ):
            # --- constants ---
            ident = const_pool.tile([P, P], F32)
            make_identity(nc, ident)
            c128 = const_pool.tile([P, P], F32)  # c128[t, l*16+r] = l
            for l in range(MAX_LORAS):
                nc.vector.memset(c128[:, l * RANK:(l + 1) * RANK], float(l))
            slot_sb = const_pool.tile([P, NT], F32)  # col i = token tile i
            nc.sync.dma_start(
                slot_sb[:], slotf.rearrange("(n p) o -> p (n o)", p=P))

            # --- resident inputs ---
            xsb = xpool.tile([P, KC * T_CORE], F32R)  # free idx = k*T_CORE + t
            for k in range(KC):
                nc.sync.dma_start(
                    xsb[:, k * T_CORE:(k + 1) * T_CORE],
                    xT[k * P:(k + 1) * P, :])
            asb = apool.tile([P, KC * 3 * GR], F32R)  # free idx = k*384 + (g*128+l*16+r)
            for k in range(KC):
                nc.sync.dma_start(
                    asb[:, k * 3 * GR:(k + 1) * 3 * GR],
                    aT[k * P:(k + 1) * P, :])
            bqsb = bpool.tile([P, Q_SIZE], F32R)
            bksb = bpool.tile([P, KV_SIZE], F32R)
            bvsb = bpool.tile([P, KV_SIZE], F32R)
            nc.sync.dma_start(bqsb[:], bq[:, :])
            nc.sync.dma_start(bksb[:], bk[:, :])
            nc.sync.dma_start(bvsb[:], bv[:, :])

            # --- phase 1: h = x@A^T per token tile, mask, transpose ---
            # hT_sb[:, i*384 + g*128 : ...] = (mask * h_g)^T for token tile i
            hT_sb = htpool.tile([P, NT * 3 * GR], F32R)
            for i in range(NT):
                hps = hpsum.tile([P, 3 * GR], F32)
                for k in range(KC):
                    nc.tensor.matmul(
                        hps[:],
                        lhsT=xsb[:, k * T_CORE + i * P: k * T_CORE + (i + 1) * P],
                        rhs=asb[:, k * 3 * GR:(k + 1) * 3 * GR],
                        start=(k == 0), stop=(k == KC - 1))
                m128 = mpool.tile([P, P], F32)  # one-hot routing mask
                nc.vector.tensor_scalar(
                    m128[:], c128[:], slot_sb[:, i:i + 1], None,
                    op0=mybir.AluOpType.is_equal)
                hm = hmpool.tile([P, 3 * GR], F32)
                for g in range(3):
                    nc.vector.tensor_tensor(
                        hm[:, g * GR:(g + 1) * GR],
                        hps[:, g * GR:(g + 1) * GR], m128[:],
                        op=mybir.AluOpType.mult)
                for g in range(3):
                    tp = tpsum.tile([P, P], F32)
                    nc.tensor.transpose(tp[:], hm[:, g * GR:(g + 1) * GR], ident[:])
                    nc.vector.tensor_copy(
                        hT_sb[:, (i * 3 + g) * GR:(i * 3 + g + 1) * GR], tp[:])

            # --- phase 2: y = x@W^T + hm@B, streamed over 512-wide o-chunks ---
            for j in range(OJ):
                if j < Q_SIZE // 512:
                    g, bsl = 0, bqsb[:, j * 512:(j + 1) * 512]
                elif j == Q_SIZE // 512:
                    g, bsl = 1, bksb[:]
                else:
                    g, bsl = 2, bvsb[:]
                wh = []
                for half in range(2):
                    wsb = wpool.tile([P, 8 * 512], F32R, tag="w")
                    for kk in range(8):
                        k = half * 8 + kk
                        nc.sync.dma_start(
                            wsb[:, kk * 512:(kk + 1) * 512],
                            wT[k * P:(k + 1) * P, j * 512:(j + 1) * 512])
                    wh.append(wsb)
                for i in range(NT):
                    ops = opsum.tile([P, 512], F32)
                    for k in range(KC):
                        nc.tensor.matmul(
                            ops[:],
                            lhsT=xsb[:, k * T_CORE + i * P: k * T_CORE + (i + 1) * P],
                            rhs=wh[k // 8][:, (k % 8) * 512:(k % 8 + 1) * 512],
                            start=(k == 0), stop=False)
                    nc.tensor.matmul(
                        ops[:],
                        lhsT=hT_sb[:, (i * 3 + g) * GR:(i * 3 + g + 1) * GR],
                        rhs=bsl,
                        start=False, stop=True)
                    osb = opool.tile([P, 512], F32)
                    nc.scalar.copy(osb[:], ops[:])
                    nc.sync.dma_start(
                        y[i * P:(i + 1) * P, j * 512:(j + 1) * 512], osb[:])
    nc.compile()
    return nc


def prep_in_maps(x, weight, lora_A, lora_B_q, lora_B_k, lora_B_v,
                 lora_scaling, token_to_slot):
    x = np.asarray(x, dtype=np.float32)
    weight = np.asarray(weight, dtype=np.float32)
    lora_A = np.asarray(lora_A, dtype=np.float32)
    lora_B_q = np.asarray(lora_B_q, dtype=np.float32)
    lora_B_k = np.asarray(lora_B_k, dtype=np.float32)
    lora_B_v = np.asarray(lora_B_v, dtype=np.float32)
    lora_scaling = np.asarray(lora_scaling, dtype=np.float32)
    slot = np.asarray(token_to_slot)

    xT = np.ascontiguousarray(x.T)                      # (2048, 8192)
    wT = np.ascontiguousarray(weight.T)                 # (2048, 3072)
    # aT col = g*128 + l*16 + r
    aT = np.ascontiguousarray(
        lora_A.transpose(1, 0, 2, 3).reshape(3 * GR, HIDDEN).T)
    # b row = l*16 + r, with scaling folded in
    bq = np.ascontiguousarray(
        (lora_scaling[:, None, None] * lora_B_q).transpose(0, 2, 1).reshape(GR, Q_SIZE))
    bk = np.ascontiguousarray(
        (lora_scaling[:, None, None] * lora_B_k).transpose(0, 2, 1).reshape(GR, KV_SIZE))
    bv = np.ascontiguousarray(
        (lora_scaling[:, None, None] * lora_B_v).transpose(0, 2, 1).reshape(GR, KV_SIZE))
    slotf = slot.astype(np.float32).reshape(T, 1)

    in_maps = []
    for c in range(N_CORES):
        in_maps.append({
            "xT": np.ascontiguousarray(xT[:, c * T_CORE:(c + 1) * T_CORE]),
            "wT": wT,
            "aT": aT,
            "bq": bq,
            "bk": bk,
            "bv": bv,
            "slotf": np.ascontiguousarray(slotf[c * T_CORE:(c + 1) * T_CORE]),
        })
    return in_maps


def kernel(**inputs):
    from concourse.bass_utils import run_bass_kernel_spmd
    if "nc" not in _NC_CACHE:
        _NC_CACHE["nc"] = build_nc()
    nc = _NC_CACHE["nc"]
    in_maps = prep_in_maps(**inputs)
    res = run_bass_kernel_spmd(nc, in_maps, core_ids=list(range(N_CORES)))
    return np.concatenate([r["y"] for r in res.results], axis=0)

